# revision 1
# baseline (speedup 1.0000x reference)
"""Brute-force kNN graph (N=65536, D=3, k=12) on 8 Trainium2 NeuronCores.

Device (per core, rows sharded 8 x 8192):
  - PE computes s[p, f] = 2*x_row[p].x_col[f] - ||x_col[f]||^2 via K=4
    augmented matmuls (stationary = [2x | -1], moving = [x | xsq]).
    s = ||x_row||^2 - dist, so top-s == nearest.
  - ACT evacuates PSUM -> SBUF (two 2048-col chunks per 4096 window).
  - DVE runs 2 levels of pairwise tensor_max (2 reads/cycle) shrinking each
    4096 window to 1024 group-maxes, then max8 + max_index (1 read/cycle)
    take the top-8 groups: 128 group positions per row naming 512 candidate
    columns. These cover the global top-13 (12 neighbours + self) unless one
    window holds >=9 of the top-13 in distinct groups -- probability
    ~1.7e-7 per row for index-random neighbours, and verified exhaustively
    against the reference on the actual fixed dataset.
Host:
  - rescores all 512 candidates per row with arithmetic that mimics the
    XLA-CPU reference (fma-style fp32 dot emulated via fp64), applies the
    self penalty, and takes the top-12 with lax.top_k's lowest-index-first
    tie-break via a packed (dist_bits, id) int64 key.
"""

import os
import sys

import numpy as np

for _p in ("/root/.axon_site/_ro/trn_rl_repo", "/opt/trn_rl_repo"):
    try:
        import concourse  # noqa: F401

        break
    except ImportError:
        if os.path.isdir(_p) and _p not in sys.path:
            sys.path.append(_p)

import concourse.bacc as bacc
import concourse.mybir as mybir
import concourse.tile as tile
from concourse.bass_utils import run_bass_kernel_spmd

F32 = mybir.dt.float32
U16 = mybir.dt.uint16

K_OUT = 12
SELF_MASK = np.float32(1e9)
CHUNK = 2048


def build_knn_nc(N, R, QW=16384, WIN=4096, LV=2):
    """WIN: window width (max 16384). PSUM chunks of 2048 are copied by ACT
    into a WIN-wide SBUF tile. LV levels of DVE pairwise tensor_max (each
    reads 2 elems/cycle via both SBUF ports) shrink the window 2^LV-fold
    before the 1x-rate max8+max_index scans; positions then name groups of
    2^LV adjacent columns, which the host expands and rescores."""
    assert N % QW == 0 and QW % CHUNK == 0 and R % 128 == 0
    assert WIN % CHUNK == 0 and QW % WIN == 0
    cpw = WIN // CHUNK  # psum chunks per window
    nq = N // QW
    ncq = QW // CHUNK
    nwq = QW // WIN  # windows per quarter
    nwin = N // WIN
    nblk = R // 128
    NC = nwin * 8  # candidate slots per row

    nc = bacc.Bacc(None, target_bir_lowering=False, debug=False)
    xcols = nc.dram_tensor("xcols", [4, N], F32, kind="ExternalInput")
    xrows = nc.dram_tensor("xrows", [4, R], F32, kind="ExternalInput")
    out_pos = nc.dram_tensor("out_pos", [R, NC], U16, kind="ExternalOutput")

    with tile.TileContext(nc) as tc:
        with (
            tc.tile_pool(name="const", bufs=1) as cpool,
            tc.tile_pool(name="xcq", bufs=1) as xcq_pool,
            tc.tile_pool(name="cand", bufs=1) as cand_pool,
            tc.tile_pool(name="sbig", bufs=2) as sbig_pool,
            tc.tile_pool(name="mx", bufs=6) as mx_pool,
            tc.tile_pool(name="psum", bufs=2, space="PSUM") as psum_pool,
        ):
            xr_sb = cpool.tile([128, R], F32, tag="xr")
            nc.gpsimd.dma_start(out=xr_sb[0:4, :], in_=xrows[:, :])
            cpos = cand_pool.tile([128, nblk * NC], U16, tag="cpos")

            for q in range(nq):
                xcq = xcq_pool.tile([128, QW], F32, tag="xcq")
                nc.gpsimd.dma_start(
                    out=xcq[0:4, :], in_=xcols[:, q * QW : (q + 1) * QW]
                )
                for blk in range(nblk):
                    lhsT = xr_sb[0:4, blk * 128 : (blk + 1) * 128]
                    for w in range(nwq):
                        sb = sbig_pool.tile([128, WIN], F32, tag="sb")
                        for cc in range(cpw):
                            c = w * cpw + cc
                            ps = psum_pool.tile([128, CHUNK], F32, tag="ps")
                            for m in range(4):
                                nc.tensor.matmul(
                                    ps[:, m * 512 : (m + 1) * 512],
                                    lhsT,
                                    xcq[0:4, c * CHUNK + m * 512 : c * CHUNK + (m + 1) * 512],
                                    start=True,
                                    stop=True,
                                )
                            nc.scalar.activation(
                                out=sb[:, cc * CHUNK : (cc + 1) * CHUNK],
                                in_=ps[:, :],
                                func=mybir.ActivationFunctionType.Copy,
                            )
                        scan = sb
                        width = WIN
                        for lv in range(LV):
                            v = scan.rearrange("p (n t) -> p n t", t=2)
                            pm = sbig_pool.tile(
                                [128, width // 2], F32, tag=f"pm{lv}"
                            )
                            nc.vector.tensor_max(pm[:, :], v[:, :, 0], v[:, :, 1])
                            scan = pm
                            width //= 2
                        mx = mx_pool.tile([128, 8], F32, tag="mx")
                        nc.vector.max(out=mx[:, :], in_=scan[:, :])
                        co = blk * NC + (q * nwq + w) * 8
                        nc.vector.max_index(
                            out=cpos[:, co : co + 8],
                            in_max=mx[:, :],
                            in_values=scan[:, :],
                        )

            for blk in range(nblk):
                nc.gpsimd.dma_start(
                    out=out_pos[blk * 128 : (blk + 1) * 128, :],
                    in_=cpos[:, blk * NC : (blk + 1) * NC],
                )

    nc.compile()
    return nc


def host_prep(x, n_cores):
    x = np.ascontiguousarray(np.asarray(x, dtype=np.float32))
    N = x.shape[0]
    R = N // n_cores
    xsq = ((x[:, 0] * x[:, 0] + x[:, 1] * x[:, 1]) + x[:, 2] * x[:, 2]).astype(
        np.float32
    )
    xcols = np.ascontiguousarray(
        np.concatenate([x.T, xsq[None, :]], axis=0).astype(np.float32)
    )
    in_maps = []
    for i in range(n_cores):
        rows = slice(i * R, (i + 1) * R)
        xr = np.ascontiguousarray(
            np.concatenate(
                [2.0 * x[rows].T, np.full((1, R), -1.0, np.float32)], axis=0
            ).astype(np.float32)
        )
        in_maps.append({"xcols": xcols, "xrows": xr})
    return in_maps, xsq


def host_finish(x, xsq, pos_all, k, win=4096, expand=1):
    """Rescore candidates with XLA-CPU-style fp32 arithmetic (fma dot
    emulated via fp64) and take the stable top-k. pos_all[:, i*8:(i+1)*8]
    hold window i's top-8 positions at group granularity `expand` (each
    position names `expand` adjacent columns, all rescored)."""
    N = pos_all.shape[0]  # rows 0..N map to x[0:N]
    nslot = pos_all.shape[1]
    cbase = ((np.arange(nslot, dtype=np.int32) // 8) * (win // expand)).astype(
        np.int32
    )
    gid0 = pos_all.astype(np.int32) + cbase[None, :]
    if expand > 1:
        gid = (
            gid0[:, :, None] * expand + np.arange(expand, dtype=np.int32)
        ).reshape(N, nslot * expand)
    else:
        gid = gid0

    out_d = np.empty((N, k), np.float32)
    out_i = np.empty((N, k), np.int32)
    CB = 4096
    xsq64 = xsq.astype(np.float64)
    x0, x1, x2 = x[:, 0], x[:, 1], x[:, 2]

    def _do(s):
        e = min(s + CB, N)
        g = gid[s:e]  # (cb, NCAND)
        # m = fma(a2,b2, fma(a1,b1, fl(a0*b0))) in fp32, emulated in fp64
        m = (x0[s:e, None].astype(np.float64) * x0[g]).astype(np.float32)
        m = (x1[s:e, None].astype(np.float64) * x1[g] + m).astype(np.float32)
        m = (x2[s:e, None].astype(np.float64) * x2[g] + m).astype(np.float32)
        A = (xsq64[s:e][:, None] + xsq64[g]).astype(np.float32)
        dist = (A.astype(np.float64) - 2.0 * m.astype(np.float64)).astype(
            np.float32
        )
        np.maximum(dist, 0.0, out=dist)
        np.add(dist, 0.0, out=dist)  # flush -0.0 to +0.0 for bit-monotone keys
        rows = np.arange(s, e, dtype=np.int32)[:, None]
        # pack (dist, gid) into one int64 key: dist >= 0 so its bit pattern
        # is order-monotone; gid < 2^17 breaks ties lowest-id-first, exactly
        # like lax.top_k. Self entries get the max key (ref adds 1e9).
        key = dist.view(np.uint32).astype(np.int64) * 131072 + g
        key[g == rows] = np.int64(1) << 62
        sel = np.argpartition(key, k, axis=1)[:, :k]
        skey = np.take_along_axis(key, sel, axis=1)
        o = np.argsort(skey, axis=1)
        skey = np.take_along_axis(skey, o, axis=1)
        out_i[s:e] = (skey & 131071).astype(np.int32)
        out_d[s:e] = (
            (skey >> 17).astype(np.uint32).view(np.float32).astype(np.float32)
        )

    from concurrent.futures import ThreadPoolExecutor

    with ThreadPoolExecutor(max_workers=8) as ex:
        list(ex.map(_do, range(0, N, CB)))
    return out_d, out_i


_NC_CACHE = {}


def kernel(x, k, chunk_size):
    n_cores = 8
    x = np.ascontiguousarray(np.asarray(x, dtype=np.float32))
    N = x.shape[0]
    R = N // n_cores
    key = (N, R)
    if key not in _NC_CACHE:
        _NC_CACHE[key] = build_knn_nc(N, R)
    nc = _NC_CACHE[key]
    in_maps, xsq = host_prep(x, n_cores)
    res = run_bass_kernel_spmd(nc, in_maps, list(range(n_cores)))
    pos_all = np.concatenate(
        [res.results[i]["out_pos"] for i in range(n_cores)], axis=0
    )
    return host_finish(x, xsq, pos_all, int(k), win=4096, expand=4)



# revision 13
# speedup vs baseline: 24.0205x; 24.0205x over previous
"""Brute-force-free kNN graph (N=65536, D=3, k=12) on 8 Trainium2 NeuronCores.

Host sorts points along a Morton curve over rank-quantized coordinates, so
spatial neighbours land close in sorted order. Each 128-row block then only
scores a B=4096-wide window of sorted columns (vs all 65536 brute force):

Device (per core, 8192 sorted rows = 64 blocks):
  - PE computes s[p, j] = 2*x_row[p].x_col[j] - ||x_col[j]||^2 via K=4
    fp32r matmuls (1 cy/row at free size 512); s = ||x_row||^2 - dist.
  - A 4-level pairwise tensor_max tree (DVE takes level 0 from PSUM, Pool
    the SBUF mid-levels) reduces each window to 256 group-maxes (group g =
    window cols {g + 256*m}, 16 cols per group).
  - Group maxes stream back to DRAM; no on-device top-k at all.
Host:
  - picks the top-32 groups per row (a true neighbour's group can be
    outranked only by the <=12 better points, so its group ranks <=13 --
    margin 19 absorbs fp32r score rounding), rescores all 512 named
    columns with XLA-CPU-exact fp32 arithmetic, takes the stable top-12.
  - a sound grid certificate (ball of the found 12th distance must be
    covered by window-resident Morton cells) flags rows whose neighbours
    may fall outside the window; those get an exact host fallback.
"""

import os
import sys

import numpy as np

for _p in ("/root/.axon_site/_ro/trn_rl_repo", "/opt/trn_rl_repo"):
    try:
        import concourse  # noqa: F401

        break
    except ImportError:
        if os.path.isdir(_p) and _p not in sys.path:
            sys.path.append(_p)

import concourse.bacc as bacc
import concourse.mybir as mybir
import concourse.tile as tile
from concourse.bass_utils import run_bass_kernel_spmd

import ml_dtypes

BF16NP = np.dtype(ml_dtypes.bfloat16)

F32 = mybir.dt.float32
BF16 = mybir.dt.bfloat16

K_OUT = 12
N_CORES = 8
B = 4096  # window width per 128-row block
G = 16  # columns per group (window reduced to B/G group maxes)
TOPG = 32  # groups rescored per row (host-side selection)
PAD = B // 2 - 64  # sentinel padding each side of the sorted array
SENT_XY = 1.0e4  # sentinel coordinate (pads never win: score ~ -3e8)
SENT_SQ = 3.0e8


def build_knn_nc(R, W):
    """R rows per core, W = R - 128 + B moving columns (padded coords)."""
    assert R % 128 == 0
    nblk = R // 128
    NG = B // G  # group maxes per block (256)

    nc = bacc.Bacc(None, target_bir_lowering=False, debug=False)
    xw_d = nc.dram_tensor("xw", [36, W], BF16, kind="ExternalInput")
    xr_d = nc.dram_tensor("xr", [36, R], BF16, kind="ExternalInput")
    gm_d = nc.dram_tensor("gm", [R, NG], F32, kind="ExternalOutput")

    with tile.TileContext(nc) as tc:
        with (
            tc.tile_pool(name="const", bufs=1) as cpool,
            tc.tile_pool(name="t0p", bufs=2) as t0p,
            tc.tile_pool(name="t1p", bufs=2) as t1p,
            tc.tile_pool(name="t2p", bufs=2) as t2p,
            tc.tile_pool(name="t3p", bufs=2) as t3p,
            tc.tile_pool(name="gmp", bufs=3) as gmp,
            tc.tile_pool(name="psum", bufs=1, space="PSUM") as psum_pool,
        ):
            xw = cpool.tile([128, W], BF16, tag="xw")
            xr = cpool.tile([128, R], BF16, tag="xr")
            nc.scalar.dma_start(out=xw[0:36, :], in_=xw_d[:, :])
            nc.scalar.dma_start(out=xr[0:36, :], in_=xr_d[:, :])

            for blk in range(nblk):
                lhsT = xr[0:36, blk * 128 : (blk + 1) * 128]
                base = blk * 128
                psA = psum_pool.tile([128, 2048], F32, tag="psA")
                psB = psum_pool.tile([128, 2048], F32, tag="psB")
                for m in range(4):
                    nc.tensor.matmul(
                        psA[:, m * 512 : (m + 1) * 512],
                        lhsT,
                        xw[0:36, base + m * 512 : base + (m + 1) * 512],
                        start=True,
                        stop=True,
                    )
                for m in range(4):
                    nc.tensor.matmul(
                        psB[:, m * 512 : (m + 1) * 512],
                        lhsT,
                        xw[0:36, base + 2048 + m * 512 : base + 2048 + (m + 1) * 512],
                        start=True,
                        stop=True,
                    )
                # ACT evacuates chunk A to SBUF; DVE pairs it against chunk B
                # (tensor_tensor allows only one PSUM operand). L0 pairs col
                # j with col j+2048, so after L3 group g = {g + 256*m, m<16}.
                t0 = t0p.tile([128, 2048], F32, tag="t0")
                nc.scalar.activation(
                    out=t0[:, :], in_=psA[:, :],
                    func=mybir.ActivationFunctionType.Copy,
                )
                t1 = t1p.tile([128, 2048], F32, tag="t1")
                nc.vector.tensor_max(t1[:, :], t0[:, :], psB[:, :])
                t2 = t2p.tile([128, 1024], F32, tag="t2")
                nc.vector.tensor_max(t2[:, :], t1[:, 0:1024], t1[:, 1024:2048])
                t3 = t3p.tile([128, 512], F32, tag="t3")
                nc.vector.tensor_max(t3[:, :], t2[:, 0:512], t2[:, 512:1024])
                # L3 (DVE) -> 256 group maxes
                gm = gmp.tile([128, NG], F32, tag="gm")
                nc.vector.tensor_max(gm[:, :], t3[:, 0:256], t3[:, 256:512])
                nc.scalar.dma_start(
                    out=gm_d[blk * 128 : (blk + 1) * 128, :], in_=gm[:, :]
                )

    nc.compile()
    return nc


# ---------------------------------------------------------------- host side


def _morton3(q):
    def part1by2(v):
        v = v.astype(np.uint64)
        v = (v | (v << np.uint64(32))) & np.uint64(0x1F00000000FFFF)
        v = (v | (v << np.uint64(16))) & np.uint64(0x1F0000FF0000FF)
        v = (v | (v << np.uint64(8))) & np.uint64(0x100F00F00F00F00F)
        v = (v | (v << np.uint64(4))) & np.uint64(0x10C30C30C30C30C3)
        v = (v | (v << np.uint64(2))) & np.uint64(0x1249249249249249)
        return v

    return part1by2(q[:, 0]) | (part1by2(q[:, 1]) << np.uint64(1)) | (
        part1by2(q[:, 2]) << np.uint64(2)
    )


def _bf16x3(a):
    """Split fp32 (4, n) into three bf16 planes summing ~exactly to a."""
    a0 = a.astype(BF16NP)
    r1 = (a - a0.astype(np.float32)).astype(np.float32)
    a1 = r1.astype(BF16NP)
    a2 = (r1 - a1.astype(np.float32)).astype(BF16NP)
    return a0, a1, a2


def host_prep(x):
    """Sort rows by Morton code of per-dim ranks; build padded device inputs.

    Scores are computed on-device as a K=36 bf16 matmul: stationary rows
    w = [2x | -1] and moving rows m = [x | xsq] are each split into three
    bf16 planes (w0+w1+w2 ~= w exactly); all 9 cross products accumulate in
    fp32 PSUM, giving ~fp32-accurate scores at bf16 PE throughput.
    """
    N = x.shape[0]
    R = N // N_CORES
    W = R - 128 + B
    ranks = np.empty((N, 3), np.uint64)
    for d in range(3):
        ranks[np.argsort(x[:, d], kind="stable"), d] = np.arange(N, dtype=np.uint64)
    order = np.argsort(_morton3(ranks), kind="stable").astype(np.int64)
    xs = x[order]  # (N, 3) sorted
    xsqs = (
        (xs[:, 0] * xs[:, 0] + xs[:, 1] * xs[:, 1]) + xs[:, 2] * xs[:, 2]
    ).astype(np.float32)

    NP = N + 2 * PAD
    xp = np.full((4, NP), SENT_XY, np.float32)
    xp[0:3, PAD : PAD + N] = xs.T
    xp[3, :] = SENT_SQ
    xp[3, PAD : PAD + N] = xsqs
    m0, m1, m2 = _bf16x3(xp)
    # moving K-blocks (i, j) lexicographic: block t uses m_{t%3}
    xw_full = np.concatenate([m0, m1, m2, m0, m1, m2, m0, m1, m2], axis=0)

    in_maps = []
    for c in range(N_CORES):
        rows = slice(c * R, (c + 1) * R)
        w = np.concatenate(
            [2.0 * xs[rows].T, np.full((1, R), -1.0, np.float32)], axis=0
        ).astype(np.float32)
        w0, w1, w2 = _bf16x3(w)
        # stationary block t uses w_{t//3}
        xr = np.concatenate([w0, w0, w0, w1, w1, w1, w2, w2, w2], axis=0)
        xw = np.ascontiguousarray(xw_full[:, c * R : c * R + W])
        in_maps.append({"xw": xw, "xr": np.ascontiguousarray(xr)})
    return in_maps, order, ranks


def _exact_rescore(x, xsq64, gid, rows_orig):
    """XLA-CPU-exact distances for candidate ids gid (M, C); returns packed
    (dist_bits, id) int64 keys (self/invalid get the max key)."""
    x0, x1, x2 = x[:, 0], x[:, 1], x[:, 2]
    r = rows_orig
    m = (x0[r, None].astype(np.float64) * x0[gid]).astype(np.float32)
    m = (x1[r, None].astype(np.float64) * x1[gid] + m).astype(np.float32)
    m = (x2[r, None].astype(np.float64) * x2[gid] + m).astype(np.float32)
    A = (xsq64[r][:, None] + xsq64[gid]).astype(np.float32)
    dist = (A.astype(np.float64) - 2.0 * m.astype(np.float64)).astype(np.float32)
    np.maximum(dist, 0.0, out=dist)
    np.add(dist, 0.0, out=dist)  # flush -0.0 for bit-monotone keys
    key = dist.view(np.uint32).astype(np.int64) * 131072 + gid
    key[gid == r[:, None]] = np.int64(1) << 62
    return key


def _topk_from_keys(key, k):
    sel = np.argpartition(key, k, axis=1)[:, :k]
    skey = np.take_along_axis(key, sel, axis=1)
    o = np.argsort(skey, axis=1)
    skey = np.take_along_axis(skey, o, axis=1)
    idx = (skey & 131071).astype(np.int32)
    dist = (skey >> 17).astype(np.uint32).view(np.float32).astype(np.float32)
    return dist, idx


def host_finish(x, gm_all, order, ranks, k):
    """Select top groups, rescore exactly, certify, fall back where needed."""
    from concurrent.futures import ThreadPoolExecutor

    N = x.shape[0]
    # fp32 stepwise like XLA-CPU (each square and add rounded to fp32)
    xsq64 = (
        (x[:, 0] * x[:, 0] + x[:, 1] * x[:, 1]) + x[:, 2] * x[:, 2]
    ).astype(np.float32).astype(np.float64)

    # --- candidate ids per sorted row: TOPG groups of G columns
    NG = B // G
    sel = np.argpartition(-gm_all, TOPG, axis=1)[:, :TOPG]  # (N, TOPG) group ids
    srow = np.arange(N, dtype=np.int64)
    wbase = (srow // 128) * 128  # window start, padded coords
    # padded col = wbase + group + 256*m
    pcol = (
        wbase[:, None, None]
        + sel[:, :, None]
        + (np.arange(G, dtype=np.int64) * NG)[None, None, :]
    ).reshape(N, TOPG * G)
    spos = pcol - PAD  # sorted position
    valid = (spos >= 0) & (spos < N)

    gid = np.empty((N, TOPG * G), np.int32)
    rows_orig = order.astype(np.int32)  # sorted row -> original id
    np.copyto(gid, rows_orig[:, None])  # invalid -> self (masked by key rule)
    gid[valid] = order[spos[valid].astype(np.int64)].astype(np.int32)

    out_d = np.empty((N, k), np.float32)
    out_i = np.empty((N, k), np.int32)

    CB = 4096

    def _do(s):
        e = min(s + CB, N)
        key = _exact_rescore(x, xsq64, gid[s:e], rows_orig[s:e])
        d, i = _topk_from_keys(key, k)
        out_d[rows_orig[s:e]] = d
        out_i[rows_orig[s:e]] = i

    with ThreadPoolExecutor(max_workers=8) as ex:
        list(ex.map(_do, range(0, N, CB)))

    # --- certificate (in original-id space): ball(x_i, rho_i) must be
    # covered by Morton cells entirely inside row i's window.
    # out_d holds SQUARED distances; the cert ball radius is its sqrt
    rho = np.sqrt(out_d[:, k - 1].astype(np.float64)) * (1 + 1e-6) + 1e-12
    LB = 5  # cert grid: 2^LB bins per dim
    SH = 16 - LB
    pos_of = np.empty(N, np.int64)  # original id -> sorted position
    pos_of[order] = srow
    wlo = (pos_of // 128) * 128 - PAD  # window range in sorted positions
    whi = wlo + B  # exclusive

    cid_pts = _morton3((ranks >> np.uint64(SH)).astype(np.uint64)).astype(np.int64)
    NCELL = 1 << (3 * LB)
    cmin = np.full(NCELL, np.iinfo(np.int64).max, np.int64)
    cmax = np.full(NCELL, -1, np.int64)
    np.minimum.at(cmin, cid_pts, pos_of)
    np.maximum.at(cmax, cid_pts, pos_of)

    lob = np.empty((N, 3), np.int64)
    hib = np.empty((N, 3), np.int64)
    for d in range(3):
        sv = np.sort(x[:, d].astype(np.float64))
        lo = np.searchsorted(sv, x[:, d].astype(np.float64) - rho, "left")
        hi = np.searchsorted(sv, x[:, d].astype(np.float64) + rho, "right") - 1
        lob[:, d] = lo >> SH
        hib[:, d] = np.minimum(hi, N - 1) >> SH

    nb = hib - lob + 1
    MAXB = 6
    cert_ok = np.all(nb <= MAXB, axis=1)
    q = np.empty((N, 3), np.uint64)
    for dx in range(MAXB):
        for dy in range(MAXB):
            for dz in range(MAXB):
                m = (
                    cert_ok
                    & (dx < nb[:, 0])
                    & (dy < nb[:, 1])
                    & (dz < nb[:, 2])
                )
                if not m.any():
                    continue
                q[m, 0] = (lob[m, 0] + dx).astype(np.uint64)
                q[m, 1] = (lob[m, 1] + dy).astype(np.uint64)
                q[m, 2] = (lob[m, 2] + dz).astype(np.uint64)
                cell = _morton3(q[m]).astype(np.int64)
                cm, cM = cmin[cell], cmax[cell]
                ok = (cm > cM) | ((cm >= wlo[m]) & (cM < whi[m]))
                mm = m.copy()
                mm[m] = ~ok
                cert_ok[mm] = False

    fb = np.where(~cert_ok)[0]
    if fb.size:
        # exact fallback: approximate distances against all points, then
        # exact rescore of the closest 64.
        xf = x[fb]
        d2 = (
            xsq64[fb][:, None]
            + xsq64[None, :]
            - 2.0 * (xf.astype(np.float64) @ x.T.astype(np.float64))
        ).astype(np.float32)
        d2[np.arange(fb.size), fb] = np.inf
        cand = np.argpartition(d2, 64, axis=1)[:, :64].astype(np.int32)
        key = _exact_rescore(x, xsq64, cand, fb.astype(np.int32))
        d, i = _topk_from_keys(key, k)
        out_d[fb] = d
        out_i[fb] = i
    return out_d, out_i


_NC_CACHE = {}


def kernel(x, k, chunk_size):
    x = np.ascontiguousarray(np.asarray(x, dtype=np.float32))
    N = x.shape[0]
    R = N // N_CORES
    W = R - 128 + B
    key = (N, R)
    if key not in _NC_CACHE:
        _NC_CACHE[key] = build_knn_nc(R, W)
    nc = _NC_CACHE[key]
    in_maps, order, ranks = host_prep(x)
    res = run_bass_kernel_spmd(nc, in_maps, list(range(N_CORES)))
    gm_all = np.concatenate(
        [res.results[c]["gm"] for c in range(N_CORES)], axis=0
    )
    return host_finish(x, gm_all, order, ranks, int(k))


# revision 17
# speedup vs baseline: 46.5446x; 1.9377x over previous
"""Brute-force-free kNN graph (N=65536, D=3, k=12) on 8 Trainium2 NeuronCores.

Host sorts points along a Morton curve over rank-quantized coordinates, so
spatial neighbours land close in sorted order. Each 128-row block then only
scores a B=4096-wide window of sorted columns (vs all 65536 brute force):

Device (per core, 8192 sorted rows = 64 blocks):
  - PE computes s[p, j] = 2*x_row[p].x_col[j] - ||x_col[j]||^2 via K=4
    fp32r matmuls (1 cy/row at free size 512); s = ||x_row||^2 - dist.
  - A 4-level pairwise tensor_max tree (DVE takes level 0 from PSUM, Pool
    the SBUF mid-levels) reduces each window to 256 group-maxes (group g =
    window cols {g + 256*m}, 16 cols per group).
  - Group maxes stream back to DRAM; no on-device top-k at all.
Host:
  - picks the top-32 groups per row (a true neighbour's group can be
    outranked only by the <=12 better points, so its group ranks <=13 --
    margin 19 absorbs fp32r score rounding), rescores all 512 named
    columns with XLA-CPU-exact fp32 arithmetic, takes the stable top-12.
  - a sound grid certificate (ball of the found 12th distance must be
    covered by window-resident Morton cells) flags rows whose neighbours
    may fall outside the window; those get an exact host fallback.
"""

import os
import sys

import numpy as np

for _p in ("/root/.axon_site/_ro/trn_rl_repo", "/opt/trn_rl_repo"):
    try:
        import concourse  # noqa: F401

        break
    except ImportError:
        if os.path.isdir(_p) and _p not in sys.path:
            sys.path.append(_p)

import concourse.bacc as bacc
import concourse.mybir as mybir
import concourse.tile as tile
from concourse.bass_utils import run_bass_kernel_spmd

import ml_dtypes

BF16NP = np.dtype(ml_dtypes.bfloat16)

F32 = mybir.dt.float32
BF16 = mybir.dt.bfloat16

K_OUT = 12
N_CORES = 8
B = 2048  # window width per 128-row block
G = 8  # columns per group (window reduced to B/G group maxes)
TOPG = 32  # groups rescored per row (host-side selection)
PAD = B // 2 - 64  # sentinel padding each side of the sorted array
SENT_XY = 1.0e4  # sentinel coordinate (pads never win: score ~ -3e8)
SENT_SQ = 3.0e8


def build_knn_nc(R, W):
    """R rows per core, W = R - 128 + B moving columns (padded coords)."""
    assert R % 128 == 0
    nblk = R // 128
    NG = B // G  # group maxes per block (256)

    nc = bacc.Bacc(None, target_bir_lowering=False, debug=False)
    xw_d = nc.dram_tensor("xw", [36, W], BF16, kind="ExternalInput")
    xr_d = nc.dram_tensor("xr", [36, R], BF16, kind="ExternalInput")
    gm_d = nc.dram_tensor("gm", [R, NG], F32, kind="ExternalOutput")

    with tile.TileContext(nc) as tc:
        with (
            tc.tile_pool(name="const", bufs=1) as cpool,
            tc.tile_pool(name="t0p", bufs=2) as t0p,
            tc.tile_pool(name="t1p", bufs=2) as t1p,
            tc.tile_pool(name="t2p", bufs=2) as t2p,
            tc.tile_pool(name="gmp", bufs=3) as gmp,
            tc.tile_pool(name="psum", bufs=2, space="PSUM") as psum_pool,
        ):
            xw = cpool.tile([128, W], BF16, tag="xw")
            xr = cpool.tile([128, R], BF16, tag="xr")
            nc.scalar.dma_start(out=xw[0:36, :], in_=xw_d[:, :])
            nc.scalar.dma_start(out=xr[0:36, :], in_=xr_d[:, :])

            for blk in range(nblk):
                lhsT = xr[0:36, blk * 128 : (blk + 1) * 128]
                base = blk * 128
                ps = psum_pool.tile([128, 2048], F32, tag="ps")
                for m in range(4):
                    nc.tensor.matmul(
                        ps[:, m * 512 : (m + 1) * 512],
                        lhsT,
                        xw[0:36, base + m * 512 : base + (m + 1) * 512],
                        start=True,
                        stop=True,
                    )
                # ACT evacuates the left half to SBUF; DVE pairs it against
                # the right half (tensor_tensor allows only one PSUM operand).
                # L0 pairs col j with j+1024, L1 j/j+512, L2 j/j+256, so
                # group g = {g + 256*m, m<8}.
                t0 = t0p.tile([128, 1024], F32, tag="t0")
                nc.scalar.activation(
                    out=t0[:, :], in_=ps[:, 0:1024],
                    func=mybir.ActivationFunctionType.Copy,
                )
                t1 = t1p.tile([128, 1024], F32, tag="t1")
                nc.vector.tensor_max(t1[:, :], t0[:, :], ps[:, 1024:2048])
                t2 = t2p.tile([128, 512], F32, tag="t2")
                nc.vector.tensor_max(t2[:, :], t1[:, 0:512], t1[:, 512:1024])
                gm = gmp.tile([128, NG], F32, tag="gm")
                nc.vector.tensor_max(gm[:, :], t2[:, 0:256], t2[:, 256:512])
                nc.sync.dma_start(
                    out=gm_d[blk * 128 : (blk + 1) * 128, :], in_=gm[:, :]
                )

    nc.compile()
    return nc


# ---------------------------------------------------------------- host side


def _morton3(q):
    def part1by2(v):
        v = v.astype(np.uint64)
        v = (v | (v << np.uint64(32))) & np.uint64(0x1F00000000FFFF)
        v = (v | (v << np.uint64(16))) & np.uint64(0x1F0000FF0000FF)
        v = (v | (v << np.uint64(8))) & np.uint64(0x100F00F00F00F00F)
        v = (v | (v << np.uint64(4))) & np.uint64(0x10C30C30C30C30C3)
        v = (v | (v << np.uint64(2))) & np.uint64(0x1249249249249249)
        return v

    return part1by2(q[:, 0]) | (part1by2(q[:, 1]) << np.uint64(1)) | (
        part1by2(q[:, 2]) << np.uint64(2)
    )


def _bf16x3(a):
    """Split fp32 (4, n) into three bf16 planes summing ~exactly to a."""
    a0 = a.astype(BF16NP)
    r1 = (a - a0.astype(np.float32)).astype(np.float32)
    a1 = r1.astype(BF16NP)
    a2 = (r1 - a1.astype(np.float32)).astype(BF16NP)
    return a0, a1, a2


def host_prep(x):
    """Sort rows by Morton code of per-dim ranks; build padded device inputs.

    Scores are computed on-device as a K=36 bf16 matmul: stationary rows
    w = [2x | -1] and moving rows m = [x | xsq] are each split into three
    bf16 planes (w0+w1+w2 ~= w exactly); all 9 cross products accumulate in
    fp32 PSUM, giving ~fp32-accurate scores at bf16 PE throughput.
    """
    N = x.shape[0]
    R = N // N_CORES
    W = R - 128 + B
    ranks = np.empty((N, 3), np.uint64)
    for d in range(3):
        ranks[np.argsort(x[:, d], kind="stable"), d] = np.arange(N, dtype=np.uint64)
    order = np.argsort(_morton3(ranks), kind="stable").astype(np.int64)
    xs = x[order]  # (N, 3) sorted
    xsqs = (
        (xs[:, 0] * xs[:, 0] + xs[:, 1] * xs[:, 1]) + xs[:, 2] * xs[:, 2]
    ).astype(np.float32)

    NP = N + 2 * PAD
    xp = np.full((4, NP), SENT_XY, np.float32)
    xp[0:3, PAD : PAD + N] = xs.T
    xp[3, :] = SENT_SQ
    xp[3, PAD : PAD + N] = xsqs
    m0, m1, m2 = _bf16x3(xp)
    # moving K-blocks (i, j) lexicographic: block t uses m_{t%3}
    xw_full = np.concatenate([m0, m1, m2, m0, m1, m2, m0, m1, m2], axis=0)

    in_maps = []
    for c in range(N_CORES):
        rows = slice(c * R, (c + 1) * R)
        w = np.concatenate(
            [2.0 * xs[rows].T, np.full((1, R), -1.0, np.float32)], axis=0
        ).astype(np.float32)
        w0, w1, w2 = _bf16x3(w)
        # stationary block t uses w_{t//3}
        xr = np.concatenate([w0, w0, w0, w1, w1, w1, w2, w2, w2], axis=0)
        xw = np.ascontiguousarray(xw_full[:, c * R : c * R + W])
        in_maps.append({"xw": xw, "xr": np.ascontiguousarray(xr)})
    return in_maps, order, ranks


def _exact_rescore(x, xsq64, gid, rows_orig):
    """XLA-CPU-exact distances for candidate ids gid (M, C); returns packed
    (dist_bits, id) int64 keys (self/invalid get the max key)."""
    x0, x1, x2 = x[:, 0], x[:, 1], x[:, 2]
    r = rows_orig
    m = (x0[r, None].astype(np.float64) * x0[gid]).astype(np.float32)
    m = (x1[r, None].astype(np.float64) * x1[gid] + m).astype(np.float32)
    m = (x2[r, None].astype(np.float64) * x2[gid] + m).astype(np.float32)
    A = (xsq64[r][:, None] + xsq64[gid]).astype(np.float32)
    dist = (A.astype(np.float64) - 2.0 * m.astype(np.float64)).astype(np.float32)
    np.maximum(dist, 0.0, out=dist)
    np.add(dist, 0.0, out=dist)  # flush -0.0 for bit-monotone keys
    key = dist.view(np.uint32).astype(np.int64) * 131072 + gid
    key[gid == r[:, None]] = np.int64(1) << 62
    return key


def _topk_from_keys(key, k):
    sel = np.argpartition(key, k, axis=1)[:, :k]
    skey = np.take_along_axis(key, sel, axis=1)
    o = np.argsort(skey, axis=1)
    skey = np.take_along_axis(skey, o, axis=1)
    idx = (skey & 131071).astype(np.int32)
    dist = (skey >> 17).astype(np.uint32).view(np.float32).astype(np.float32)
    return dist, idx


def host_finish(x, gm_all, order, ranks, k):
    """Select top groups, rescore exactly, certify, fall back where needed."""
    from concurrent.futures import ThreadPoolExecutor

    N = x.shape[0]
    # fp32 stepwise like XLA-CPU (each square and add rounded to fp32)
    xsq64 = (
        (x[:, 0] * x[:, 0] + x[:, 1] * x[:, 1]) + x[:, 2] * x[:, 2]
    ).astype(np.float32).astype(np.float64)

    # --- candidate ids per sorted row: TOPG groups of G columns
    NG = B // G
    sel = np.argpartition(-gm_all, TOPG, axis=1)[:, :TOPG]  # (N, TOPG) group ids
    srow = np.arange(N, dtype=np.int64)
    wbase = (srow // 128) * 128  # window start, padded coords
    # padded col = wbase + group + 256*m
    pcol = (
        wbase[:, None, None]
        + sel[:, :, None]
        + (np.arange(G, dtype=np.int64) * NG)[None, None, :]
    ).reshape(N, TOPG * G)
    spos = pcol - PAD  # sorted position
    valid = (spos >= 0) & (spos < N)

    gid = np.empty((N, TOPG * G), np.int32)
    rows_orig = order.astype(np.int32)  # sorted row -> original id
    np.copyto(gid, rows_orig[:, None])  # invalid -> self (masked by key rule)
    gid[valid] = order[spos[valid].astype(np.int64)].astype(np.int32)

    out_d = np.empty((N, k), np.float32)
    out_i = np.empty((N, k), np.int32)

    CB = 4096

    def _do(s):
        e = min(s + CB, N)
        key = _exact_rescore(x, xsq64, gid[s:e], rows_orig[s:e])
        d, i = _topk_from_keys(key, k)
        out_d[rows_orig[s:e]] = d
        out_i[rows_orig[s:e]] = i

    with ThreadPoolExecutor(max_workers=8) as ex:
        list(ex.map(_do, range(0, N, CB)))

    # --- certificate (in original-id space): ball(x_i, rho_i) must be
    # covered by Morton cells entirely inside row i's window.
    # out_d holds SQUARED distances; the cert ball radius is its sqrt
    rho = np.sqrt(out_d[:, k - 1].astype(np.float64)) * (1 + 1e-6) + 1e-12
    LB = 5  # cert grid: 2^LB bins per dim
    SH = 16 - LB
    pos_of = np.empty(N, np.int64)  # original id -> sorted position
    pos_of[order] = srow
    wlo = (pos_of // 128) * 128 - PAD  # window range in sorted positions
    whi = wlo + B  # exclusive

    cid_pts = _morton3((ranks >> np.uint64(SH)).astype(np.uint64)).astype(np.int64)
    NCELL = 1 << (3 * LB)
    cmin = np.full(NCELL, np.iinfo(np.int64).max, np.int64)
    cmax = np.full(NCELL, -1, np.int64)
    np.minimum.at(cmin, cid_pts, pos_of)
    np.maximum.at(cmax, cid_pts, pos_of)

    lob = np.empty((N, 3), np.int64)
    hib = np.empty((N, 3), np.int64)
    for d in range(3):
        sv = np.sort(x[:, d].astype(np.float64))
        lo = np.searchsorted(sv, x[:, d].astype(np.float64) - rho, "left")
        hi = np.searchsorted(sv, x[:, d].astype(np.float64) + rho, "right") - 1
        lob[:, d] = lo >> SH
        hib[:, d] = np.minimum(hi, N - 1) >> SH

    nb = hib - lob + 1
    MAXB = 6
    cert_ok = np.all(nb <= MAXB, axis=1)
    q = np.empty((N, 3), np.uint64)
    for dx in range(MAXB):
        for dy in range(MAXB):
            for dz in range(MAXB):
                m = (
                    cert_ok
                    & (dx < nb[:, 0])
                    & (dy < nb[:, 1])
                    & (dz < nb[:, 2])
                )
                if not m.any():
                    continue
                q[m, 0] = (lob[m, 0] + dx).astype(np.uint64)
                q[m, 1] = (lob[m, 1] + dy).astype(np.uint64)
                q[m, 2] = (lob[m, 2] + dz).astype(np.uint64)
                cell = _morton3(q[m]).astype(np.int64)
                cm, cM = cmin[cell], cmax[cell]
                ok = (cm > cM) | ((cm >= wlo[m]) & (cM < whi[m]))
                mm = m.copy()
                mm[m] = ~ok
                cert_ok[mm] = False

    fb = np.where(~cert_ok)[0]
    LAST_STATS["fallback_rows"] = int(fb.size)
    if fb.size:
        # exact fallback: approximate distances against all points, then
        # exact rescore of the closest 64.
        xf = x[fb]
        d2 = (
            xsq64[fb][:, None]
            + xsq64[None, :]
            - 2.0 * (xf.astype(np.float64) @ x.T.astype(np.float64))
        ).astype(np.float32)
        d2[np.arange(fb.size), fb] = np.inf
        cand = np.argpartition(d2, 64, axis=1)[:, :64].astype(np.int32)
        key = _exact_rescore(x, xsq64, cand, fb.astype(np.int32))
        d, i = _topk_from_keys(key, k)
        out_d[fb] = d
        out_i[fb] = i
    return out_d, out_i


_NC_CACHE = {}
LAST_STATS = {}


def kernel(x, k, chunk_size):
    x = np.ascontiguousarray(np.asarray(x, dtype=np.float32))
    N = x.shape[0]
    R = N // N_CORES
    W = R - 128 + B
    key = (N, R)
    if key not in _NC_CACHE:
        _NC_CACHE[key] = build_knn_nc(R, W)
    nc = _NC_CACHE[key]
    in_maps, order, ranks = host_prep(x)
    res = run_bass_kernel_spmd(nc, in_maps, list(range(N_CORES)))
    gm_all = np.concatenate(
        [res.results[c]["gm"] for c in range(N_CORES)], axis=0
    )
    return host_finish(x, gm_all, order, ranks, int(k))


# revision 22
# speedup vs baseline: 53.4647x; 1.1487x over previous
"""Brute-force-free kNN graph (N=65536, D=3, k=12) on 8 Trainium2 NeuronCores.

Host sorts points along a Morton curve over rank-quantized coordinates, so
spatial neighbours land close in sorted order. Each 128-row block then only
scores a B=4096-wide window of sorted columns (vs all 65536 brute force):

Device (per core, 8192 sorted rows = 64 blocks):
  - PE computes s[p, j] = 2*x_row[p].x_col[j] - ||x_col[j]||^2 via K=4
    fp32r matmuls (1 cy/row at free size 512); s = ||x_row||^2 - dist.
  - A 4-level pairwise tensor_max tree (DVE takes level 0 from PSUM, Pool
    the SBUF mid-levels) reduces each window to 256 group-maxes (group g =
    window cols {g + 256*m}, 16 cols per group).
  - Group maxes stream back to DRAM; no on-device top-k at all.
Host:
  - picks the top-32 groups per row (a true neighbour's group can be
    outranked only by the <=12 better points, so its group ranks <=13 --
    margin 19 absorbs fp32r score rounding), rescores all 512 named
    columns with XLA-CPU-exact fp32 arithmetic, takes the stable top-12.
  - a sound grid certificate (ball of the found 12th distance must be
    covered by window-resident Morton cells) flags rows whose neighbours
    may fall outside the window; those get an exact host fallback.
"""

import os
import sys

import numpy as np

for _p in ("/root/.axon_site/_ro/trn_rl_repo", "/opt/trn_rl_repo"):
    try:
        import concourse  # noqa: F401

        break
    except ImportError:
        if os.path.isdir(_p) and _p not in sys.path:
            sys.path.append(_p)

import concourse.bacc as bacc
import concourse.mybir as mybir
import concourse.tile as tile
from concourse.bass_utils import run_bass_kernel_spmd

import ml_dtypes

BF16NP = np.dtype(ml_dtypes.bfloat16)

F32 = mybir.dt.float32
BF16 = mybir.dt.bfloat16

K_OUT = 12
N_CORES = 8
B = 2048  # window width per 128-row block
G = 4  # columns per group (window reduced to B/G group maxes)
TOPG = 32  # groups rescored per row (host-side selection)
PAD = B // 2 - 64  # sentinel padding each side of the sorted array
SENT_XY = 1.0e4  # sentinel coordinate (pads never win: score ~ -3e8)
SENT_SQ = 3.0e8


def build_knn_nc(R, W):
    """R rows per core, W = R - 128 + B moving columns (padded coords)."""
    assert R % 128 == 0
    nblk = R // 128
    NG = B // G  # group maxes per block (256)

    nc = bacc.Bacc(None, target_bir_lowering=False, debug=False)
    xw_d = nc.dram_tensor("xw", [36, W], BF16, kind="ExternalInput")
    xr_d = nc.dram_tensor("xr", [36, R], BF16, kind="ExternalInput")
    gm_d = nc.dram_tensor("gm", [R, NG], F32, kind="ExternalOutput")

    with tile.TileContext(nc) as tc:
        with (
            tc.tile_pool(name="const", bufs=1) as cpool,
            tc.tile_pool(name="t0p", bufs=2) as t0p,
            tc.tile_pool(name="t1p", bufs=2) as t1p,
            tc.tile_pool(name="gmp", bufs=3) as gmp,
            tc.tile_pool(name="psum", bufs=2, space="PSUM") as psum_pool,
        ):
            xw = cpool.tile([128, W], BF16, tag="xw")
            xr = cpool.tile([128, R], BF16, tag="xr")
            nc.scalar.dma_start(out=xw[0:36, :], in_=xw_d[:, :])
            nc.scalar.dma_start(out=xr[0:36, :], in_=xr_d[:, :])

            for blk in range(nblk):
                lhsT = xr[0:36, blk * 128 : (blk + 1) * 128]
                base = blk * 128
                ps = psum_pool.tile([128, 2048], F32, tag="ps")
                for m in range(4):
                    nc.tensor.matmul(
                        ps[:, m * 512 : (m + 1) * 512],
                        lhsT,
                        xw[0:36, base + m * 512 : base + (m + 1) * 512],
                        start=True,
                        stop=True,
                    )
                # ACT evacuates the left half to SBUF; DVE pairs it against
                # the right half (tensor_tensor allows only one PSUM operand).
                # L0 pairs col j with j+1024, L1 j/j+512, so group
                # g = {g + 512*m, m<4}.
                t0 = t0p.tile([128, 1024], F32, tag="t0")
                nc.scalar.activation(
                    out=t0[:, :], in_=ps[:, 0:1024],
                    func=mybir.ActivationFunctionType.Copy,
                )
                t1 = t1p.tile([128, 1024], F32, tag="t1")
                nc.vector.tensor_max(t1[:, :], t0[:, :], ps[:, 1024:2048])
                gm = gmp.tile([128, NG], F32, tag="gm")
                nc.vector.tensor_max(gm[:, :], t1[:, 0:512], t1[:, 512:1024])
                nc.sync.dma_start(
                    out=gm_d[blk * 128 : (blk + 1) * 128, :], in_=gm[:, :]
                )

    nc.compile()
    return nc


# ---------------------------------------------------------------- host side


def _morton3(q):
    def part1by2(v):
        v = v.astype(np.uint64)
        v = (v | (v << np.uint64(32))) & np.uint64(0x1F00000000FFFF)
        v = (v | (v << np.uint64(16))) & np.uint64(0x1F0000FF0000FF)
        v = (v | (v << np.uint64(8))) & np.uint64(0x100F00F00F00F00F)
        v = (v | (v << np.uint64(4))) & np.uint64(0x10C30C30C30C30C3)
        v = (v | (v << np.uint64(2))) & np.uint64(0x1249249249249249)
        return v

    return part1by2(q[:, 0]) | (part1by2(q[:, 1]) << np.uint64(1)) | (
        part1by2(q[:, 2]) << np.uint64(2)
    )


def _bf16x3(a):
    """Split fp32 (4, n) into three bf16 planes summing ~exactly to a."""
    a0 = a.astype(BF16NP)
    r1 = (a - a0.astype(np.float32)).astype(np.float32)
    a1 = r1.astype(BF16NP)
    a2 = (r1 - a1.astype(np.float32)).astype(BF16NP)
    return a0, a1, a2


def host_prep(x):
    """Sort rows by Morton code of per-dim ranks; build padded device inputs.

    Scores are computed on-device as a K=36 bf16 matmul: stationary rows
    w = [2x | -1] and moving rows m = [x | xsq] are each split into three
    bf16 planes (w0+w1+w2 ~= w exactly); all 9 cross products accumulate in
    fp32 PSUM, giving ~fp32-accurate scores at bf16 PE throughput.
    """
    N = x.shape[0]
    R = N // N_CORES
    W = R - 128 + B
    ranks = np.empty((N, 3), np.uint64)
    for d in range(3):
        ranks[np.argsort(x[:, d], kind="stable"), d] = np.arange(N, dtype=np.uint64)
    order = np.argsort(_morton3(ranks), kind="stable").astype(np.int64)
    xs = x[order]  # (N, 3) sorted
    xsqs = (
        (xs[:, 0] * xs[:, 0] + xs[:, 1] * xs[:, 1]) + xs[:, 2] * xs[:, 2]
    ).astype(np.float32)

    NP = N + 2 * PAD
    xp = np.full((4, NP), SENT_XY, np.float32)
    xp[0:3, PAD : PAD + N] = xs.T
    xp[3, :] = SENT_SQ
    xp[3, PAD : PAD + N] = xsqs
    m0, m1, m2 = _bf16x3(xp)
    # moving K-blocks (i, j) lexicographic: block t uses m_{t%3}
    xw_full = np.concatenate([m0, m1, m2, m0, m1, m2, m0, m1, m2], axis=0)

    in_maps = []
    for c in range(N_CORES):
        rows = slice(c * R, (c + 1) * R)
        w = np.concatenate(
            [2.0 * xs[rows].T, np.full((1, R), -1.0, np.float32)], axis=0
        ).astype(np.float32)
        w0, w1, w2 = _bf16x3(w)
        # stationary block t uses w_{t//3}
        xr = np.concatenate([w0, w0, w0, w1, w1, w1, w2, w2, w2], axis=0)
        xw = np.ascontiguousarray(xw_full[:, c * R : c * R + W])
        in_maps.append({"xw": xw, "xr": np.ascontiguousarray(xr)})
    return in_maps, order, ranks


def _exact_rescore(x, xsq64, gid, rows_orig):
    """XLA-CPU-exact distances for candidate ids gid (M, C); returns packed
    (dist_bits, id) int64 keys (self/invalid get the max key)."""
    x0, x1, x2 = x[:, 0], x[:, 1], x[:, 2]
    r = rows_orig
    m = (x0[r, None].astype(np.float64) * x0[gid]).astype(np.float32)
    m = (x1[r, None].astype(np.float64) * x1[gid] + m).astype(np.float32)
    m = (x2[r, None].astype(np.float64) * x2[gid] + m).astype(np.float32)
    A = (xsq64[r][:, None] + xsq64[gid]).astype(np.float32)
    dist = (A.astype(np.float64) - 2.0 * m.astype(np.float64)).astype(np.float32)
    np.maximum(dist, 0.0, out=dist)
    np.add(dist, 0.0, out=dist)  # flush -0.0 for bit-monotone keys
    key = dist.view(np.uint32).astype(np.int64) * 131072 + gid
    key[gid == r[:, None]] = np.int64(1) << 62
    return key


def _topk_from_keys(key, k):
    sel = np.argpartition(key, k, axis=1)[:, :k]
    skey = np.take_along_axis(key, sel, axis=1)
    o = np.argsort(skey, axis=1)
    skey = np.take_along_axis(skey, o, axis=1)
    idx = (skey & 131071).astype(np.int32)
    dist = (skey >> 17).astype(np.uint32).view(np.float32).astype(np.float32)
    return dist, idx


def host_finish(x, gm_all, order, ranks, k):
    """Select top groups, rescore exactly, certify, fall back where needed."""
    from concurrent.futures import ThreadPoolExecutor

    N = x.shape[0]
    # fp32 stepwise like XLA-CPU (each square and add rounded to fp32)
    xsq64 = (
        (x[:, 0] * x[:, 0] + x[:, 1] * x[:, 1]) + x[:, 2] * x[:, 2]
    ).astype(np.float32).astype(np.float64)

    # --- candidate ids per sorted row: TOPG groups of G columns
    NG = B // G
    sel = np.argpartition(-gm_all, TOPG, axis=1)[:, :TOPG]  # (N, TOPG) group ids
    srow = np.arange(N, dtype=np.int64)
    wbase = (srow // 128) * 128  # window start, padded coords
    # padded col = wbase + group + 256*m
    pcol = (
        wbase[:, None, None]
        + sel[:, :, None]
        + (np.arange(G, dtype=np.int64) * NG)[None, None, :]
    ).reshape(N, TOPG * G)
    spos = pcol - PAD  # sorted position
    valid = (spos >= 0) & (spos < N)

    gid = np.empty((N, TOPG * G), np.int32)
    rows_orig = order.astype(np.int32)  # sorted row -> original id
    np.copyto(gid, rows_orig[:, None])  # invalid -> self (masked by key rule)
    gid[valid] = order[spos[valid].astype(np.int64)].astype(np.int32)

    out_d = np.empty((N, k), np.float32)
    out_i = np.empty((N, k), np.int32)

    CB = 4096

    def _do(s):
        e = min(s + CB, N)
        key = _exact_rescore(x, xsq64, gid[s:e], rows_orig[s:e])
        d, i = _topk_from_keys(key, k)
        out_d[rows_orig[s:e]] = d
        out_i[rows_orig[s:e]] = i

    with ThreadPoolExecutor(max_workers=8) as ex:
        list(ex.map(_do, range(0, N, CB)))

    # --- certificate (in original-id space): ball(x_i, rho_i) must be
    # covered by Morton cells entirely inside row i's window.
    # out_d holds SQUARED distances; the cert ball radius is its sqrt
    rho = np.sqrt(out_d[:, k - 1].astype(np.float64)) * (1 + 1e-6) + 1e-12
    LB = 5  # cert grid: 2^LB bins per dim
    SH = 16 - LB
    pos_of = np.empty(N, np.int64)  # original id -> sorted position
    pos_of[order] = srow
    wlo = (pos_of // 128) * 128 - PAD  # window range in sorted positions
    whi = wlo + B  # exclusive

    cid_pts = _morton3((ranks >> np.uint64(SH)).astype(np.uint64)).astype(np.int64)
    NCELL = 1 << (3 * LB)
    cmin = np.full(NCELL, np.iinfo(np.int64).max, np.int64)
    cmax = np.full(NCELL, -1, np.int64)
    np.minimum.at(cmin, cid_pts, pos_of)
    np.maximum.at(cmax, cid_pts, pos_of)

    lob = np.empty((N, 3), np.int64)
    hib = np.empty((N, 3), np.int64)
    for d in range(3):
        sv = np.sort(x[:, d].astype(np.float64))
        lo = np.searchsorted(sv, x[:, d].astype(np.float64) - rho, "left")
        hi = np.searchsorted(sv, x[:, d].astype(np.float64) + rho, "right") - 1
        lob[:, d] = lo >> SH
        hib[:, d] = np.minimum(hi, N - 1) >> SH

    nb = hib - lob + 1
    MAXB = 6
    cert_ok = np.all(nb <= MAXB, axis=1)
    q = np.empty((N, 3), np.uint64)
    for dx in range(MAXB):
        for dy in range(MAXB):
            for dz in range(MAXB):
                m = (
                    cert_ok
                    & (dx < nb[:, 0])
                    & (dy < nb[:, 1])
                    & (dz < nb[:, 2])
                )
                if not m.any():
                    continue
                q[m, 0] = (lob[m, 0] + dx).astype(np.uint64)
                q[m, 1] = (lob[m, 1] + dy).astype(np.uint64)
                q[m, 2] = (lob[m, 2] + dz).astype(np.uint64)
                cell = _morton3(q[m]).astype(np.int64)
                cm, cM = cmin[cell], cmax[cell]
                ok = (cm > cM) | ((cm >= wlo[m]) & (cM < whi[m]))
                mm = m.copy()
                mm[m] = ~ok
                cert_ok[mm] = False

    fb = np.where(~cert_ok)[0]
    LAST_STATS["fallback_rows"] = int(fb.size)
    if fb.size:
        # exact fallback: fp32 approximate distances against all points
        # (chunked), then exact rescore of the closest 64 per row.
        xsq32 = xsq64.astype(np.float32)
        FCB = 256

        def _fb_do(s):
            e = min(s + FCB, fb.size)
            rows = fb[s:e]
            d2 = -2.0 * (x[rows] @ x.T)
            d2 += xsq32[rows][:, None]
            d2 += xsq32[None, :]
            d2[np.arange(rows.size), rows] = np.inf
            cand = np.argpartition(d2, 64, axis=1)[:, :64].astype(np.int32)
            key = _exact_rescore(x, xsq64, cand, rows.astype(np.int32))
            d, i = _topk_from_keys(key, k)
            out_d[rows] = d
            out_i[rows] = i

        with ThreadPoolExecutor(max_workers=8) as ex:
            list(ex.map(_fb_do, range(0, fb.size, FCB)))
    return out_d, out_i


_NC_CACHE = {}
LAST_STATS = {}


def kernel(x, k, chunk_size):
    x = np.ascontiguousarray(np.asarray(x, dtype=np.float32))
    N = x.shape[0]
    R = N // N_CORES
    W = R - 128 + B
    key = (N, R)
    if key not in _NC_CACHE:
        _NC_CACHE[key] = build_knn_nc(R, W)
    nc = _NC_CACHE[key]
    in_maps, order, ranks = host_prep(x)
    res = run_bass_kernel_spmd(nc, in_maps, list(range(N_CORES)))
    gm_all = np.concatenate(
        [res.results[c]["gm"] for c in range(N_CORES)], axis=0
    )
    return host_finish(x, gm_all, order, ranks, int(k))
